# revision 5
# baseline (speedup 1.0000x reference)
"""Trainium2 Bass kernel for nn_Attention_41729902248209.

8-head attention block: x (8, 512, 32, 32) -> QKV proj -> softmax attention
-> out proj + residual. Data-parallel over batch: one batch element per
NeuronCore (8 cores).

Per-core dataflow (n = 1024 tokens, cin = 512, H = 8 heads, D = 64):
  - everything stays "transposed" (feature dim on partitions) so no on-chip
    transposes are needed anywhere:
      qT, kT : (f' = 64h+d on partitions, n free)   [head pairs share a tile]
      v      : (n on partitions, 65h+d free, with a ones column per head)
      scoresT: (j on partitions, i free) = k @ qT   [2 heads via tile_position]
      pT     : exp(scoresT) in fp16 (no max subtraction; logits are O(7))
      outT~  : [v | 1].T @ pT -> (65, i): rows 0:64 = unscaled outT, row 64 =
               softmax denominator (the ones column integrates exp for free)
      yT     : W_last.T.T @ outT_scaled + (x + b_last)  (residual, fp32)
  - softmax scale 1/8 is folded into W_q host-side; b_last is folded into the
    residual; b_q/b_k are per-partition DVE adds; b_v is a DVE tensor add.
  - denominators: rows extracted from PSUM partition 64, packed via SBUF->SBUF
    DMA, one batched reciprocal, replicated across partitions with GPSIMD
    partition_broadcast, applied as an fp16 DVE multiply.
"""

import numpy as np

import concourse.mybir as mybir
import concourse.tile as tile
from concourse import bacc
from concourse.bass_utils import run_bass_kernel_spmd

F16 = mybir.dt.float16
F32 = mybir.dt.float32

BS = 8
H = 8
D = 64
CIN = 512
N = 1024
NK = CIN // 128  # contraction tiles for cin
NJT = N // 128  # j tiles
NCH = N // 512  # i chunks of 512
VROW = H * (D + 1)  # 520: per j-tile v row: 8x[v_h (64) | 1]

AF = mybir.ActivationFunctionType
ALU = mybir.AluOpType


def _emit(tc, d, sb, ps):
    nc = tc.nc

    x16_sb = sb.tile([128, NK * N], F16, tag="x16")
    xr_sb = sb.tile([128, NK * N], F32, tag="xr")
    wq_sb = sb.tile([128, NK * 512], F16, tag="wq")
    wk_sb = sb.tile([128, NK * 512], F16, tag="wk")
    wv_sb = sb.tile([128, NK * 512], F16, tag="wv")
    wl_sb = sb.tile([128, NK * 512], F16, tag="wl")
    bqk_sb = sb.tile([128, 8], F32, tag="bqk")
    bvb_sb = sb.tile([128, 512], F32, tag="bvb")
    qT_sb = sb.tile([128, 4 * N], F16, tag="qT")
    kT_sb = sb.tile([128, 4 * N], F16, tag="kT")
    v_sb = sb.tile([128, NJT * VROW], F16, tag="v")
    ou_sb = sb.tile([128, 4 * N], F16, tag="outT_u")
    os_sb = sb.tile([128, 4 * N], F16, tag="outT_s")
    dscr_sb = sb.tile([128, 16 * 512], F32, tag="dscr")

    # --- input DMAs (ktile k of a (512, W) dram tensor -> cols [W*k, W*k+W)) ---
    for k in range(NK):
        r = slice(128 * k, 128 * k + 128)
        nc.sync.dma_start(wq_sb[:, 512 * k : 512 * k + 512], d["wq"].ap()[r, :])
        nc.sync.dma_start(wk_sb[:, 512 * k : 512 * k + 512], d["wk"].ap()[r, :])
        nc.sync.dma_start(x16_sb[:, N * k : N * k + N], d["x16"].ap()[r, :])
    nc.sync.dma_start(bqk_sb[:], d["bqk"].ap())
    for k in range(NK):
        r = slice(128 * k, 128 * k + 128)
        nc.sync.dma_start(wv_sb[:, 512 * k : 512 * k + 512], d["wv"].ap()[r, :])
    nc.sync.dma_start(bvb_sb[:], d["bvb"].ap())
    # ones columns for v~ (overwritten by v evacs except the 65th columns)
    nc.vector.memset(v_sb[:], 1.0)
    for k in range(NK):
        r = slice(128 * k, 128 * k + 128)
        nc.sync.dma_start(wl_sb[:, 512 * k : 512 * k + 512], d["wl"].ap()[r, :])
        nc.sync.dma_start(xr_sb[:, N * k : N * k + N], d["xr"].ap()[r, :])

    # --- stage emitters ---
    def qk_tile(t):
        """Project q and k for f'-tile t (heads 2t, 2t+1), with bias."""
        for wsb, dst, bcol in ((wq_sb, qT_sb, t), (wk_sb, kT_sb, 4 + t)):
            for c in range(NCH):
                p = ps.tile([128, 512], F32, tag="mm")
                for k in range(NK):
                    nc.tensor.matmul(
                        p[:],
                        wsb[:, 512 * k + 128 * t : 512 * k + 128 * t + 128],
                        x16_sb[:, N * k + 512 * c : N * k + 512 * c + 512],
                        start=(k == 0),
                        stop=(k == NK - 1),
                    )
                nc.vector.tensor_scalar_add(
                    dst[:, N * t + 512 * c : N * t + 512 * c + 512],
                    p[:],
                    bqk_sb[:, bcol : bcol + 1],
                )

    def v_tile(jt):
        """Project v for token tile jt: (128 tokens, 512 feats) + b_v."""
        p = ps.tile([128, 512], F32, tag="mm")
        for k in range(NK):
            nc.tensor.matmul(
                p[:],
                x16_sb[:, N * k + 128 * jt : N * k + 128 * jt + 128],
                wv_sb[:, 512 * k : 512 * k + 512],
                start=(k == 0),
                stop=(k == NK - 1),
            )
        dst = (
            v_sb[:, VROW * jt : VROW * jt + VROW]
            .rearrange("p (h e) -> p h e", e=D + 1)[:, :, 0:D]
        )
        nc.vector.tensor_tensor(
            dst,
            p[:].rearrange("p (h e) -> p h e", e=D),
            bvb_sb[:].rearrange("p (h e) -> p h e", e=D),
            ALU.add,
        )

    pt_tiles = {}

    def scores_exp(h):
        """scoresT (j, i) for head h + exp -> pT fp16. Heads pair via row tiles."""
        pr, hh = divmod(h, 2)
        pT = sbuf_pt_pool.tile([128, NJT * N], F16, tag="pt")
        pt_tiles[h] = pT
        for jt in range(NJT):
            sp = ps.tile([128, N], F32, tag="score", bufs=2)
            for c in range(NCH):
                nc.tensor.matmul(
                    sp[:, 512 * c : 512 * c + 512],
                    kT_sb[64 * hh : 64 * hh + 64, N * pr + 128 * jt : N * pr + 128 * jt + 128],
                    qT_sb[64 * hh : 64 * hh + 64, N * pr + 512 * c : N * pr + 512 * c + 512],
                    start=True,
                    stop=True,
                    tile_position=(64 * hh, 0),
                )
            nc.scalar.activation(pT[:, N * jt : N * jt + N], sp[:], AF.Exp)

    def attnv(h):
        """outT~ = [v_h | 1].T @ pT_h: (65, 512) per chunk; row 64 = denom."""
        pr, hh = divmod(h, 2)
        pT = pt_tiles.pop(h)
        for c in range(NCH):
            p = ps.tile([128, 512], F32, tag="mm")
            for jt in range(NJT):
                nc.tensor.matmul(
                    p[0:65, :],
                    v_sb[:, VROW * jt + 65 * h : VROW * jt + 65 * h + 65],
                    pT[:, N * jt + 512 * c : N * jt + 512 * c + 512],
                    start=(jt == 0),
                    stop=(jt == NJT - 1),
                )
            r = 2 * h + c
            # unscaled evac (fp16) + denominator row extraction
            nc.vector.tensor_copy(
                ou_sb[64 * hh : 64 * hh + 64, N * pr + 512 * c : N * pr + 512 * c + 512],
                p[0:64, :],
            )
            nc.vector.tensor_copy(dscr_sb[64:65, 512 * r : 512 * r + 512], p[64:65, :])

    def denom_batch(b):
        """Pack denom rows 8b..8b+7, reciprocal, cast to fp16, back to 1 row."""
        da = sb.tile([8, 512], F32, tag=f"dall{b}")
        nc.sync.dma_start(da[:], dscr_sb[64:65, 4096 * b : 4096 * b + 4096])
        dr = sb.tile([8, 512], F32, tag=f"drec{b}")
        nc.vector.reciprocal(dr[:], da[:])
        d16 = sb.tile([8, 512], F16, tag=f"d16_{b}")
        nc.vector.tensor_copy(d16[:], dr[:])
        # broadcast sources must start at partition 0 -> repack as one row
        drow = sb.tile([1, 4096], F16, tag=f"drow{b}")
        nc.sync.dma_start(drow[:], d16[:])
        return drow

    def scale(h, d16b):
        """outT_s = outT_u * (1/denom) via gpsimd partition broadcast + DVE."""
        pr, hh = divmod(h, 2)
        for c in range(NCH):
            r = 2 * h + c
            rb = rb_pool.tile([128, 512], F16, tag="rb")
            nc.gpsimd.partition_broadcast(
                rb[:], d16b[0:1, 512 * (r % 8) : 512 * (r % 8) + 512]
            )
            sl = slice(N * pr + 512 * c, N * pr + 512 * c + 512)
            nc.vector.tensor_tensor(
                os_sb[64 * hh : 64 * hh + 64, sl],
                ou_sb[64 * hh : 64 * hh + 64, sl],
                rb[64 * hh : 64 * hh + 64, :],
                ALU.mult,
            )

    def outproj(ct):
        """yT c-tile ct: W_lastT.T @ outT_s + (x + b_last), fp32 out + DMA."""
        for c in range(NCH):
            p = ps.tile([128, 512], F32, tag="mm")
            for k in range(NK):
                nc.tensor.matmul(
                    p[:],
                    wl_sb[:, 512 * k + 128 * ct : 512 * k + 128 * ct + 128],
                    os_sb[:, N * k + 512 * c : N * k + 512 * c + 512],
                    start=(k == 0),
                    stop=(k == NK - 1),
                )
            y = y_pool.tile([128, 512], F32, tag="y")
            nc.vector.tensor_tensor(
                y[:],
                p[:],
                xr_sb[:, N * ct + 512 * c : N * ct + 512 * c + 512],
                ALU.add,
            )
            nc.sync.dma_start(
                d["y"].ap()[128 * ct : 128 * ct + 128, 512 * c : 512 * c + 512], y[:]
            )

    # --- pools that emitters close over ---
    import contextlib

    stack = contextlib.ExitStack()
    sbuf_pt_pool = stack.enter_context(tc.tile_pool(name="pt", bufs=2))
    rb_pool = stack.enter_context(tc.tile_pool(name="rb", bufs=3))
    y_pool = stack.enter_context(tc.tile_pool(name="y", bufs=3))

    # --- software-pipelined emission (PE order: keep feeding while ACT exps) ---
    qk_tile(0)
    qk_tile(1)
    scores_exp(0)
    qk_tile(2)
    scores_exp(1)
    qk_tile(3)
    scores_exp(2)
    for jt in range(4):
        v_tile(jt)
    scores_exp(3)
    for jt in range(4, NJT):
        v_tile(jt)
    attnv(0)
    scores_exp(4)
    attnv(1)
    scores_exp(5)
    attnv(2)
    scores_exp(6)
    attnv(3)
    scores_exp(7)
    attnv(4)
    attnv(5)
    d16_0 = denom_batch(0)
    attnv(6)
    attnv(7)
    for h in range(4):
        scale(h, d16_0)
    d16_1 = denom_batch(1)
    for h in range(4, 8):
        scale(h, d16_1)
    for ct in range(4):
        outproj(ct)

    stack.close()


def _build():
    nc = bacc.Bacc("TRN2", target_bir_lowering=False, debug=False, num_devices=BS)
    d = {}
    d["x16"] = nc.dram_tensor("x16", [CIN, N], F16, kind="ExternalInput")
    d["xr"] = nc.dram_tensor("xr", [CIN, N], F32, kind="ExternalInput")
    d["wq"] = nc.dram_tensor("wq", [CIN, 512], F16, kind="ExternalInput")
    d["wk"] = nc.dram_tensor("wk", [CIN, 512], F16, kind="ExternalInput")
    d["wv"] = nc.dram_tensor("wv", [CIN, 512], F16, kind="ExternalInput")
    d["wl"] = nc.dram_tensor("wl", [CIN, 512], F16, kind="ExternalInput")
    d["bqk"] = nc.dram_tensor("bqk", [128, 8], F32, kind="ExternalInput")
    d["bvb"] = nc.dram_tensor("bvb", [128, 512], F32, kind="ExternalInput")
    d["y"] = nc.dram_tensor("y", [CIN, N], F32, kind="ExternalOutput")

    with tile.TileContext(nc) as tc:
        with (
            tc.tile_pool(name="sb", bufs=1) as sb,
            tc.tile_pool(name="ps", bufs=4, space="PSUM") as ps,
        ):
            _emit(tc, d, sb, ps)
    nc.compile()
    return nc


_NC_CACHE = {}


def get_nc():
    if "nc" not in _NC_CACHE:
        _NC_CACHE["nc"] = _build()
    return _NC_CACHE["nc"]


def host_prep(x, W_fc, b_fc, W_last, b_last):
    """Full inputs -> list of 8 per-core input maps."""
    x = np.asarray(x, dtype=np.float32)
    W_fc = np.asarray(W_fc, dtype=np.float32)
    b_fc = np.asarray(b_fc, dtype=np.float32)
    W_last = np.asarray(W_last, dtype=np.float32)
    b_last = np.asarray(b_last, dtype=np.float32)

    hh = np.arange(H).repeat(D) * 3 * D  # 192h per f'=64h+d
    dd = np.tile(np.arange(D), H)
    pq, pk, pv = hh + dd, hh + D + dd, hh + 2 * D + dd

    wq = np.ascontiguousarray((W_fc[pq] * 0.125).T).astype(np.float16)
    wk = np.ascontiguousarray(W_fc[pk].T).astype(np.float16)
    wv = np.ascontiguousarray(W_fc[pv].T).astype(np.float16)
    wl = np.ascontiguousarray(W_last.T).astype(np.float16)
    bq, bk, bv = b_fc[pq] * 0.125, b_fc[pk], b_fc[pv]
    bqk = np.ascontiguousarray(
        np.concatenate([bq.reshape(4, 128).T, bk.reshape(4, 128).T], axis=1)
    ).astype(np.float32)
    bvb = np.ascontiguousarray(np.tile(bv[None, :], (128, 1))).astype(np.float32)

    xf = x.reshape(BS, CIN, N)
    maps = []
    for b in range(BS):
        maps.append(
            {
                "x16": xf[b].astype(np.float16),
                "xr": (xf[b] + b_last[:, None]).astype(np.float32),
                "wq": wq,
                "wk": wk,
                "wv": wv,
                "wl": wl,
                "bqk": bqk,
                "bvb": bvb,
            }
        )
    return maps


def kernel(x, W_fc, b_fc, W_last, b_last):
    nc = get_nc()
    maps = host_prep(x, W_fc, b_fc, W_last, b_last)
    res = run_bass_kernel_spmd(nc, maps, core_ids=list(range(BS)))
    y = np.stack([res.results[b]["y"] for b in range(BS)])
    return y.reshape(BS, CIN, 32, 32)


# revision 35
# speedup vs baseline: 536.2471x; 536.2471x over previous
"""Trainium2 Bass kernel for nn_Attention_41729902248209.

8-head attention block: x (8, 512, 32, 32) -> QKV proj -> softmax attention
-> out proj + residual. Data-parallel over batch: one batch element per
NeuronCore (8 cores).

Per-core dataflow (n = 1024 tokens, cin = 512, H = 8 heads, D = 64):
  - everything stays "transposed" (feature dim on partitions) so no on-chip
    transposes are needed anywhere:
      qT, kT : (f' = 64h+d on partitions, n free)   [head pairs share a tile]
      v      : (n on partitions, 65h+d free, with a ones column per head)
      scoresT: (j on partitions, i free) = k @ qT   [2 heads via tile_position]
      pT     : exp(scoresT) in fp16 (no max subtraction; logits are O(7))
      outT~  : [v | 1].T @ pT -> (65, i): rows 0:64 = unscaled outT, row 64 =
               softmax denominator (the ones column integrates exp for free)
      yT     : W_last.T.T @ outT_scaled + (x + b_last)  (residual, fp32)
  - softmax scale 1/8 is folded into W_q host-side; b_last is folded into the
    residual; b_q/b_k are per-partition DVE adds; b_v is a DVE tensor add.
  - denominators (per (head, chunk)): DVE reciprocal directly off the PSUM
    denom row (partition 64), a 1-partition cross-quadrant fp16 copy to
    partition 0, GPSIMD partition_broadcast to 128 partitions, then one fp16
    DVE multiply into outT_s. No DMAs on that chain.
  - emission is software-pipelined at single-score-tile granularity: the
    attnv work is a stream of 2-matmul units drained behind the scores/exp
    stream so ACT (the 64 us exp floor) never starves while PE stays dense.
"""

import numpy as np

import concourse.mybir as mybir
import concourse.tile as tile
from concourse import bacc
from concourse.bass_utils import run_bass_kernel_spmd

F16 = mybir.dt.float16
F32 = mybir.dt.float32

BS = 8
H = 8
D = 64
CIN = 512
N = 1024
NK = CIN // 128  # contraction tiles for cin
NJT = N // 128  # j tiles
NCH = N // 512  # i chunks of 512
VROW = H * (D + 1)  # 520: per j-tile v row: 8x[v_h (64) | 1]

AF = mybir.ActivationFunctionType
NO_CONC_PROBE = False  # timing-only probe: defeat score row-group pairing
ALU = mybir.AluOpType


def _emit(tc, d, sb, ps):
    nc = tc.nc

    x16_sb = sb.tile([128, NK * N], F16, tag="x16")
    xr_sb = sb.tile([128, NK * N], F32, tag="xr")
    wq_sb = sb.tile([128, NK * 512], F16, tag="wq")
    wk_sb = sb.tile([128, NK * 512], F16, tag="wk")
    wv_sb = sb.tile([128, NK * 512], F16, tag="wv")
    wl_sb = sb.tile([128, NK * 512], F16, tag="wl")
    bqk_sb = sb.tile([128, 8], F32, tag="bqk")
    bvb_sb = sb.tile([128, 512], F32, tag="bvb")
    qT_sb = sb.tile([128, 4 * N], F16, tag="qT")
    kT_sb = sb.tile([128, 4 * N], F16, tag="kT")
    v_sb = sb.tile([128, NJT * VROW], F16, tag="v")
    ou_sb = sb.tile([128, 4 * N], F16, tag="outT_u")
    os_sb = sb.tile([128, 4 * N], F16, tag="outT_s")
    dscr_sb = sb.tile([128, 16 * 512], F32, tag="dscr")

    # --- input DMAs (ktile k of a (512, W) dram tensor -> cols [W*k, W*k+W)) ---
    # wq + x16 land first (first q-projection matmuls), then wk, then the rest
    for k in range(NK):
        r = slice(128 * k, 128 * k + 128)
        nc.sync.dma_start(wq_sb[:, 512 * k : 512 * k + 512], d["wq"].ap()[r, :])
        nc.sync.dma_start(x16_sb[:, N * k : N * k + N], d["x16"].ap()[r, :])
    for k in range(NK):
        r = slice(128 * k, 128 * k + 128)
        nc.sync.dma_start(wk_sb[:, 512 * k : 512 * k + 512], d["wk"].ap()[r, :])
    nc.sync.dma_start(bqk_sb[:], d["bqk"].ap())
    for k in range(NK):
        r = slice(128 * k, 128 * k + 128)
        nc.sync.dma_start(wv_sb[:, 512 * k : 512 * k + 512], d["wv"].ap()[r, :])
    nc.sync.dma_start(bvb_sb[:], d["bvb"].ap())
    # ones columns for v~ (column 64 of each 65-wide head block)
    ones_cols = v_sb[:].rearrange("p (jt h e) -> p jt h e", jt=NJT, e=D + 1)[
        :, :, :, D : D + 1
    ]
    nc.vector.memset(ones_cols, 1.0)
    for k in range(NK):
        r = slice(128 * k, 128 * k + 128)
        nc.sync.dma_start(wl_sb[:, 512 * k : 512 * k + 512], d["wl"].ap()[r, :])
        nc.sync.dma_start(xr_sb[:, N * k : N * k + N], d["xr"].ap()[r, :])

    # --- stage emitters ---
    def qk_tile(t):
        """Project q and k for f'-tile t (heads 2t, 2t+1), with bias."""
        for wsb, dst, bcol in ((wq_sb, qT_sb, t), (wk_sb, kT_sb, 4 + t)):
            for c in range(NCH):
                p = ps.tile([128, 512], F32, tag="mm")
                for k in range(NK):
                    nc.tensor.matmul(
                        p[:],
                        wsb[:, 512 * k + 128 * t : 512 * k + 128 * t + 128],
                        x16_sb[:, N * k + 512 * c : N * k + 512 * c + 512],
                        start=(k == 0),
                        stop=(k == NK - 1),
                    )
                nc.vector.tensor_scalar_add(
                    dst[:, N * t + 512 * c : N * t + 512 * c + 512],
                    p[:],
                    bqk_sb[:, bcol : bcol + 1],
                )

    def v_tile(jt):
        """Project v for token tile jt: (128 tokens, 512 feats) + b_v."""
        p = ps.tile([128, 512], F32, tag="mm")
        for k in range(NK):
            nc.tensor.matmul(
                p[:],
                x16_sb[:, N * k + 128 * jt : N * k + 128 * jt + 128],
                wv_sb[:, 512 * k : 512 * k + 512],
                start=(k == 0),
                stop=(k == NK - 1),
            )
        dst = (
            v_sb[:, VROW * jt : VROW * jt + VROW]
            .rearrange("p (h e) -> p h e", e=D + 1)[:, :, 0:D]
        )
        nc.vector.tensor_tensor(
            dst,
            p[:].rearrange("p (h e) -> p h e", e=D),
            bvb_sb[:].rearrange("p (h e) -> p h e", e=D),
            ALU.add,
        )

    pt_tiles = {}

    def scores_exp(h, jts):
        """scoresT (j, i) for head h + exp -> pT fp16. Heads pair via row tiles.

        Two j-tiles share one 4-bank PSUM region so exp runs as (128, 2048)
        ops (amortizes the per-op ACT overhead)."""
        pr, hh = divmod(h, 2)
        if h in pt_tiles:
            pT = pt_tiles[h]
        else:
            pT = sbuf_pt_pool.tile([128, NJT * N], F16, tag="pt", name=f"pt{h}")
            pt_tiles[h] = pT
        po = 0 if NO_CONC_PROBE else 64 * hh
        for jt in jts:
            sp = ps.tile([128, N], F32, tag="score", bufs=2)
            for c in range(NCH):
                nc.tensor.matmul(
                    sp[:, 512 * c : 512 * c + 512],
                    kT_sb[po : po + 64, N * pr + 128 * jt : N * pr + 128 * jt + 128],
                    qT_sb[po : po + 64, N * pr + 512 * c : N * pr + 512 * c + 512],
                    start=True,
                    stop=True,
                    tile_position=(po, 0),
                )
            nc.scalar.activation(pT[:, N * jt : N * jt + N], sp[:], AF.Exp)

    pv_tiles = {}

    def attnv_unit(h, c, g2):
        """2 accumulating matmuls (j-tiles 2*g2, 2*g2+1) of outT~ for (h, c);
        evac + denominator extraction after the last unit of the chunk."""
        pr, hh = divmod(h, 2)
        pT = pt_tiles[h]
        key = (h, c)
        if key not in pv_tiles:
            pv_tiles[key] = ps.tile([128, 512], F32, tag="mm", name=f"av{h}_{c}")
        p = pv_tiles[key]
        for jt in (2 * g2, 2 * g2 + 1):
            nc.tensor.matmul(
                p[0:65, :],
                v_sb[:, VROW * jt + 65 * h : VROW * jt + 65 * h + 65],
                pT[:, N * jt + 512 * c : N * jt + 512 * c + 512],
                start=(jt == 0),
                stop=(jt == NJT - 1),
            )
        if g2 == 3:
            del pv_tiles[key]
            if c == NCH - 1:
                del pt_tiles[h]
            r = 2 * h + c
            nc.vector.tensor_copy(
                ou_sb[64 * hh : 64 * hh + 64, N * pr + 512 * c : N * pr + 512 * c + 512],
                p[0:64, :],
            )
            # denominator chain, DMA-free: recip psum row (p64) -> sbuf f32
            # (p64), cross-quadrant fp16 copy p64 -> p0, gpsimd broadcast,
            # fp16 multiply into outT_s.
            dsl = dscr_sb[64:65, 512 * r : 512 * r + 512]
            nc.vector.reciprocal(dsl, p[64:65, :])
            rrow = rr_pool.tile([1, 512], F16, tag="rrow", name=f"rr{r}")
            nc.vector.tensor_copy(rrow[0:1, :], dsl)
            rb = rb_pool.tile([128, 512], F16, tag="rb", name=f"rb{r}")
            nc.gpsimd.partition_broadcast(rb[:], rrow[0:1, :])
            sl = slice(N * pr + 512 * c, N * pr + 512 * c + 512)
            nc.vector.tensor_tensor(
                os_sb[64 * hh : 64 * hh + 64, sl],
                ou_sb[64 * hh : 64 * hh + 64, sl],
                rb[64 * hh : 64 * hh + 64, :],
                ALU.mult,
            )

    def outproj(ct):
        """yT c-tile ct: W_lastT.T @ outT_s + (x + b_last), fp32 out + DMA.

        PSUM comes from the score tag's banks (idle once exps are done),
        leaving the mm tag's slots to the attnv backlog."""
        for c in range(NCH):
            p = ps.tile([128, 512], F32, tag="mm", name=f"yp{ct}_{c}")
            for k in range(NK):
                nc.tensor.matmul(
                    p[:],
                    wl_sb[:, 512 * k + 128 * ct : 512 * k + 128 * ct + 128],
                    os_sb[:, N * k + 512 * c : N * k + 512 * c + 512],
                    start=(k == 0),
                    stop=(k == NK - 1),
                )
            y = y_pool.tile([128, 512], F32, tag="y")
            nc.vector.tensor_tensor(
                y[:],
                p[:],
                xr_sb[:, N * ct + 512 * c : N * ct + 512 * c + 512],
                ALU.add,
            )
            nc.sync.dma_start(
                d["y"].ap()[128 * ct : 128 * ct + 128, 512 * c : 512 * c + 512], y[:]
            )

    # --- pools that emitters close over ---
    import contextlib

    stack = contextlib.ExitStack()
    sbuf_pt_pool = stack.enter_context(tc.tile_pool(name="pt", bufs=3))
    rb_pool = stack.enter_context(tc.tile_pool(name="rb", bufs=3))
    rr_pool = stack.enter_context(tc.tile_pool(name="rr", bufs=3))
    y_pool = stack.enter_context(tc.tile_pool(name="y", bufs=3))

    # --- software-pipelined emission (PE order: keep feeding while ACT exps) ---
    # attnv work as a stream of 2-MM units, drained behind the scores/exp
    # stream at rates that keep ACT continuously fed while PE catches up.
    av_units = [(h, c, g2) for h in range(H) for c in range(NCH) for g2 in range(4)]
    av_pos = 0

    def drain_av(n):
        nonlocal av_pos
        for _ in range(n):
            if av_pos >= len(av_units):
                return
            h, c, g2 = av_units[av_pos]
            av_pos += 1
            attnv_unit(h, c, g2)

    qk_tile(0)
    qk_tile(1)
    qk_quarters = [
        (wsb, dst, bcol, t, c)
        for t in (2, 3)
        for (wsb, dst, bcol) in ((wq_sb, qT_sb, t), (wk_sb, kT_sb, 4 + t))
        for c in range(NCH)
    ]
    for g in range(NJT):  # head 0 scores + qk tiles 2,3 (one quarter per step)
        scores_exp(0, [g])
        wsb, dst, bcol, t, c = qk_quarters[g]
        p = ps.tile([128, 512], F32, tag="mm", name=f"qk{t}_{bcol}_{c}")
        for k in range(NK):
            nc.tensor.matmul(
                p[:],
                wsb[:, 512 * k + 128 * t : 512 * k + 128 * t + 128],
                x16_sb[:, N * k + 512 * c : N * k + 512 * c + 512],
                start=(k == 0),
                stop=(k == NK - 1),
            )
        nc.vector.tensor_scalar_add(
            dst[:, N * t + 512 * c : N * t + 512 * c + 512],
            p[:],
            bqk_sb[:, bcol : bcol + 1],
        )
    for g in range(NJT):  # head 1 scores + v tiles
        scores_exp(1, [g])
        v_tile(g)
    rates = {2: 8, 3: 8, 4: 8, 5: 8, 6: 8, 7: 8}
    for h in range(2, H):
        per = [rates[h] // NJT + (1 if g < rates[h] % NJT else 0) for g in range(NJT)]
        for g in range(NJT):
            scores_exp(h, [g])
            drain_av(per[g])
    drain_av(len(av_units))  # remainder (attnv of heads 6,7 + last denoms)
    for ct in range(4):
        outproj(ct)

    stack.close()


def _build(loop=1):
    nc = bacc.Bacc("TRN2", target_bir_lowering=False, debug=False, num_devices=BS)
    d = {}
    d["x16"] = nc.dram_tensor("x16", [CIN, N], F16, kind="ExternalInput")
    d["xr"] = nc.dram_tensor("xr", [CIN, N], F32, kind="ExternalInput")
    d["wq"] = nc.dram_tensor("wq", [CIN, 512], F16, kind="ExternalInput")
    d["wk"] = nc.dram_tensor("wk", [CIN, 512], F16, kind="ExternalInput")
    d["wv"] = nc.dram_tensor("wv", [CIN, 512], F16, kind="ExternalInput")
    d["wl"] = nc.dram_tensor("wl", [CIN, 512], F16, kind="ExternalInput")
    d["bqk"] = nc.dram_tensor("bqk", [128, 8], F32, kind="ExternalInput")
    d["bvb"] = nc.dram_tensor("bvb", [128, 512], F32, kind="ExternalInput")
    d["y"] = nc.dram_tensor("y", [CIN, N], F32, kind="ExternalOutput")

    with tile.TileContext(nc) as tc:
        with (
            tc.tile_pool(name="sb", bufs=1) as sb,
            tc.tile_pool(name="ps", bufs=4, space="PSUM") as ps,
        ):
            for i in range(loop):
                if i:
                    with tc.tile_critical():
                        nc.all_engine_barrier()
                _emit(tc, d, sb, ps)
    nc.compile()
    return nc


_NC_CACHE = {}


def get_nc(loop=1):
    if loop not in _NC_CACHE:
        _NC_CACHE[loop] = _build(loop)
    return _NC_CACHE[loop]


def host_prep(x, W_fc, b_fc, W_last, b_last):
    """Full inputs -> list of 8 per-core input maps."""
    x = np.asarray(x, dtype=np.float32)
    W_fc = np.asarray(W_fc, dtype=np.float32)
    b_fc = np.asarray(b_fc, dtype=np.float32)
    W_last = np.asarray(W_last, dtype=np.float32)
    b_last = np.asarray(b_last, dtype=np.float32)

    hh = np.arange(H).repeat(D) * 3 * D  # 192h per f'=64h+d
    dd = np.tile(np.arange(D), H)
    pq, pk, pv = hh + dd, hh + D + dd, hh + 2 * D + dd

    wq = np.ascontiguousarray((W_fc[pq] * 0.125).T).astype(np.float16)
    wk = np.ascontiguousarray(W_fc[pk].T).astype(np.float16)
    wv = np.ascontiguousarray(W_fc[pv].T).astype(np.float16)
    wl = np.ascontiguousarray(W_last.T).astype(np.float16)
    bq, bk, bv = b_fc[pq] * 0.125, b_fc[pk], b_fc[pv]
    bqk = np.ascontiguousarray(
        np.concatenate([bq.reshape(4, 128).T, bk.reshape(4, 128).T], axis=1)
    ).astype(np.float32)
    bvb = np.ascontiguousarray(np.tile(bv[None, :], (128, 1))).astype(np.float32)

    xf = x.reshape(BS, CIN, N)
    maps = []
    for b in range(BS):
        maps.append(
            {
                "x16": xf[b].astype(np.float16),
                "xr": (xf[b] + b_last[:, None]).astype(np.float32),
                "wq": wq,
                "wk": wk,
                "wv": wv,
                "wl": wl,
                "bqk": bqk,
                "bvb": bvb,
            }
        )
    return maps


def kernel(x, W_fc, b_fc, W_last, b_last):
    nc = get_nc()
    maps = host_prep(x, W_fc, b_fc, W_last, b_last)
    res = run_bass_kernel_spmd(nc, maps, core_ids=list(range(BS)))
    y = np.stack([res.results[b]["y"] for b in range(BS)])
    return y.reshape(BS, CIN, 32, 32)
